# revision 1
# baseline (speedup 1.0000x reference)
"""Quantized (4-bit) LoRA linear for Trainium2, SPMD over 8 NeuronCores.

Math:  y[t,o] = sum_i x[t,i]*W[o,i] + bias[o] + 2.0 * sum_r (x@A^T)[t,r]*B[o,r]
where  W[o,i] = (nib[o,i] - zero[i]) * scale[i],  nib = unpacked 4-bit ints.

Rewrite with xs[t,i] = x[t,i]*scale[i]:
  y[t,o] = sum_i xs[t,i]*nib[o,i]        (PE matmul: fp16 xs x fp8 nib, both exact
                                          enough: nib in {0..15} is exact in fp8e4)
         + G[t,:] @ H[o,:]^T             (K=9 matmul folding LoRA + zero-correction)
         + bias[o]                       (fused into the DVE PSUM evacuation)
  G cols: 0-7 = u[t,r] = sum_i xs[t,i]*(A[r,i]/scale[i]) = (x @ A^T)[t,r],
          8   = c[t]   = sum_i xs[t,i]*zero[i]
  H rows: 0-7 = 2.0*B^T, 8 = -1

Sharding: 8-way token split (1024 tokens per core), each core computes the
full 4096 outs in two o-half passes. xs (64 KB/part) stays resident in SBUF
across both passes; the fp8 nib half (64 KB/part) streams through a shared
pool, reloaded for pass 1 pipelined behind pass 0's tail. u/G computed once
(pass 0) and reused in pass 1. All accumulation in PSUM (7+1 banks).
"""

import numpy as np

B, S, I, O = 4, 2048, 4096, 4096
T = B * S            # 8192 tokens
NCORES = 8
TC = T // NCORES     # 1024 tokens per core
OH = O // 2          # 2048 outs per pass
KC = I // 128        # 32 contraction chunks
TG = 4               # token tiles per token-group (512 tokens)
NG = TC // (TG * 128)  # 2 token groups per core

_CACHE = {}


def _build_program():
    import concourse.bacc as bacc
    import concourse.mybir as mybir
    import concourse.tile as tile

    fp16 = mybir.dt.float16
    fp32 = mybir.dt.float32
    fp8 = mybir.dt.float8e4

    nc = bacc.Bacc("TRN2", target_bir_lowering=False, debug=False)
    xsT = nc.dram_tensor("xsT", [I, TC], fp16, kind="ExternalInput")
    nibT = nc.dram_tensor("nibT", [I, O], fp8, kind="ExternalInput")
    aextT = nc.dram_tensor("aextT", [I, 9], fp16, kind="ExternalInput")
    hmat = nc.dram_tensor("hmat", [9, O], fp16, kind="ExternalInput")
    bias_bc = nc.dram_tensor("bias_bc", [128, O], fp32, kind="ExternalInput")
    y = nc.dram_tensor("y", [TC, O], fp32, kind="ExternalOutput")

    with tile.TileContext(nc) as tc:
        with (
            tc.tile_pool(name="nib", bufs=KC) as nib_pool,
            tc.tile_pool(name="consts", bufs=1) as const_pool,
            tc.tile_pool(name="xs", bufs=1) as xs_pool,
            tc.tile_pool(name="g", bufs=1) as g_pool,
            tc.tile_pool(name="out", bufs=3) as out_pool,
            tc.tile_pool(name="psum", bufs=8, space="PSUM") as psum_pool,
        ):
            h_tile = const_pool.tile([9, O], fp16, tag="h")
            bias_tile = const_pool.tile([128, O], fp32, tag="bias")
            aext_tiles = [None] * KC
            # xs resident across both passes: unique tag per tile, bufs=1
            xs_tiles = {}   # (tg, k) -> tile
            gts = [None] * NG

            for h in range(2):  # o-half pass
                o0 = h * OH
                nib_tiles = [None] * KC
                for tg in range(NG):
                    t0 = tg * TG * 128
                    # Interleave DMAs chunk-by-chunk with compute emission so
                    # the PE streams right behind the DMA (pass 0 tg 0 and the
                    # pass-1 nib reload both pipeline this way).
                    for k in range(KC):
                        if tg == 0:
                            nt = nib_pool.tile([128, OH], fp8, tag="nib",
                                               name=f"nib{h}_{k}")
                            nc.sync.dma_start(
                                nt[:], nibT[k * 128:(k + 1) * 128, o0:o0 + OH]
                            )
                            nib_tiles[k] = nt
                        if h == 0:
                            xt = xs_pool.tile([128, TG * 128], fp16,
                                              tag=f"xs{tg}_{k}", bufs=1,
                                              name=f"xs{tg}_{k}")
                            nc.sync.dma_start(
                                xt[:], xsT[k * 128:(k + 1) * 128,
                                           t0:t0 + TG * 128]
                            )
                            xs_tiles[(tg, k)] = xt
                            if tg == 0:
                                at = const_pool.tile([128, 9], fp16,
                                                     tag=f"aext{k}",
                                                     name=f"aext{k}")
                                nc.sync.dma_start(
                                    at[:], aextT[k * 128:(k + 1) * 128, :]
                                )
                                aext_tiles[k] = at
                    if h == 0 and tg == 0:
                        nc.sync.dma_start(h_tile[:], hmat[:, :])
                        nc.sync.dma_start(bias_tile[:], bias_bc[:, :])

                    # tt0's main matmuls are interleaved per-chunk with the
                    # u matmuls (pass 0) so the PE starts after chunk 0 lands.
                    ps0 = [
                        psum_pool.tile([128, 512], fp32, tag="mm",
                                       name=f"mm{h}_{tg}_0_{j}")
                        for j in range(4)
                    ]
                    if h == 0:
                        # up shares the mm pool slots (it is released by the
                        # gt copy before the 8th concurrent mm bank is needed)
                        up = psum_pool.tile([9, TG * 128], fp32, tag="mm",
                                            name=f"up{tg}")
                    for k in range(KC):
                        if h == 0:
                            nc.tensor.matmul(
                                up[:], aext_tiles[k][:], xs_tiles[(tg, k)][:],
                                start=(k == 0), stop=(k == KC - 1),
                            )
                        lhsT = xs_tiles[(tg, k)][:, 0:128]
                        for j in range(4):
                            nc.tensor.matmul(
                                ps0[j][:], lhsT,
                                nib_tiles[k][:, j * 512:(j + 1) * 512],
                                start=(k == 0), stop=False,
                            )
                    if h == 0:
                        gt = g_pool.tile([9, TG * 128], fp16, tag=f"g{tg}",
                                         bufs=1, name=f"g{tg}")
                        nc.vector.tensor_copy(gt[:, :], up[:])
                        gts[tg] = gt
                    gt = gts[tg]

                    for tt in range(TG):
                        if tt == 0:
                            ps = ps0
                        else:
                            ps = [
                                psum_pool.tile([128, 512], fp32, tag="mm",
                                               name=f"mm{h}_{tg}_{tt}_{j}")
                                for j in range(4)
                            ]
                            for k in range(KC):
                                lhsT = xs_tiles[(tg, k)][:,
                                                         tt * 128:(tt + 1) * 128]
                                for j in range(4):
                                    nc.tensor.matmul(
                                        ps[j][:], lhsT,
                                        nib_tiles[k][:, j * 512:(j + 1) * 512],
                                        start=(k == 0), stop=False,
                                    )
                        gs = gt[:, tt * 128:(tt + 1) * 128]
                        for j in range(4):
                            nc.tensor.matmul(
                                ps[j][:], gs,
                                h_tile[:, o0 + j * 512:o0 + (j + 1) * 512],
                                start=False, stop=True,
                            )
                        ot = out_pool.tile([128, OH], fp32, tag="out")
                        for j in range(4):
                            nc.vector.tensor_add(
                                ot[:, j * 512:(j + 1) * 512], ps[j][:],
                                bias_tile[:, o0 + j * 512:o0 + (j + 1) * 512],
                            )
                        trow = t0 + tt * 128
                        nc.sync.dma_start(y[trow:trow + 128, o0:o0 + OH], ot[:])
    nc.compile()
    return nc


def _prep_inputs(x, weight_quant, scale, zero, lora_A, lora_B, bias):
    """Host-side layout prep + sharding. Returns in_maps for 8 cores."""
    import ml_dtypes

    xs = (x.reshape(T, I).astype(np.float32) * scale[None, :]).astype(np.float16)
    xsT = np.ascontiguousarray(xs.T)  # [I, T]

    wq = weight_quant.astype(np.uint8)            # low byte only is populated
    nib = np.empty((O, I), np.uint8)
    nib[:, 0::2] = wq & 15
    nib[:, 1::2] = wq >> 4
    nibT = np.ascontiguousarray(nib.T.astype(ml_dtypes.float8_e4m3fn))  # [I, O]

    aextT = np.empty((I, 9), np.float16)
    aextT[:, 0:8] = (lora_A.astype(np.float32) / scale[None, :]).T
    aextT[:, 8] = zero
    aextT = np.ascontiguousarray(aextT)

    hmat = np.empty((9, O), np.float16)
    hmat[0:8, :] = 2.0 * lora_B.T
    hmat[8, :] = -1.0
    hmat = np.ascontiguousarray(hmat)
    bias_bc = np.ascontiguousarray(
        np.broadcast_to(bias.astype(np.float32), (128, O))
    )

    in_maps = []
    for c in range(NCORES):
        in_maps.append({
            "xsT": np.ascontiguousarray(xsT[:, c * TC:(c + 1) * TC]),
            "nibT": nibT,
            "aextT": aextT,
            "hmat": hmat,
            "bias_bc": bias_bc,
        })
    return in_maps


def run_on_cores(in_maps, trace=False):
    from concourse.bass_utils import run_bass_kernel_spmd

    if "nc" not in _CACHE:
        _CACHE["nc"] = _build_program()
    return run_bass_kernel_spmd(
        _CACHE["nc"], in_maps, list(range(NCORES)), trace=trace
    )


def kernel(x, weight_quant, scale, zero, lora_A, lora_B, bias):
    x = np.asarray(x)
    weight_quant = np.asarray(weight_quant)
    scale = np.asarray(scale, np.float32)
    zero = np.asarray(zero, np.float32)
    lora_A = np.asarray(lora_A, np.float32)
    lora_B = np.asarray(lora_B, np.float32)
    bias = np.asarray(bias, np.float32)

    in_maps = _prep_inputs(x, weight_quant, scale, zero, lora_A, lora_B, bias)
    res = run_on_cores(in_maps).results

    out = np.concatenate([res[c]["y"] for c in range(NCORES)], axis=0)
    return np.ascontiguousarray(out).reshape(B, S, O)



# revision 3
# speedup vs baseline: 1.8572x; 1.8572x over previous
"""Quantized (4-bit) LoRA linear for Trainium2, SPMD over 8 NeuronCores.

Math:  y[t,o] = sum_i x[t,i]*W[o,i] + bias[o] + 2.0 * sum_r (x@A^T)[t,r]*B[o,r]
where  W[o,i] = (nib[o,i] - zero[i]) * scale[i],  nib = unpacked 4-bit ints.

Rewrite with xs[t,i] = x[t,i]*scale[i], zoff = round(zero), zfrac = zero-zoff:
  y[t,o] = sum_i xs[t,i]*nib'[o,i]       nib' = nib - zoff in [-15,15], exact fp8
         + G[t,:] @ H[o,:]^T             K=10 matmul: LoRA + zfrac-corr + bias
  G cols: 0-7 = u[t,r] = (x @ A^T)[t,r], 8 = sum_i xs*zfrac, 9 = 1
  H rows: 0-7 = 2.0*B^T, 8 = -1, 9 = bias

fp8 DoubleRow: the main matmul runs in MatmulPerfMode.DoubleRow (0.5
cycles/row, 2 contraction sub-rows per partition -> 4x fp16 MAC rate).
xs is split into exact fp8 hi+lo components (xs scaled by ALPHA=256 so
both parts stay out of fp8-subnormal territory; 1/ALPHA is applied at
PSUM evacuation). Each 256-channel pair needs one hi + one lo DoubleRow
matmul -> net 2x fewer PE cycles than the fp16 baseline. G is computed
from the same fp8 hi/lo tiles (also DoubleRow); the K=10 H-apply stays
fp16. Output is written fp16 (upcast on host).

Sharding: 8-way token split (1024 tokens per core), each core computes
the full 4096 outs in two o-half passes; nib' fp8 tiles stream through a
rotating pool, xs stays resident.
"""

import numpy as np

B, S, I, O = 4, 2048, 4096, 4096
T = B * S            # 8192 tokens
NCORES = 8
TC = T // NCORES     # 1024 tokens per core
OH = O // 2          # 2048 outs per o-half pass
KP = I // 256        # 16 contraction pairs (256 channels each)
ALPHA = 256.0        # xs pre-scale so fp8 hi/lo avoid subnormals
SCALING = 2.0        # lora alpha/r

_CACHE = {}


def _build_program():
    import concourse.bacc as bacc
    import concourse.mybir as mybir
    import concourse.tile as tile

    fp16 = mybir.dt.float16
    fp32 = mybir.dt.float32
    fp8 = mybir.dt.float8e4
    DR = mybir.MatmulPerfMode.DoubleRow

    nc = bacc.Bacc("TRN2", target_bir_lowering=False, debug=False)
    # (kp, p, hl*2+s, t): hl = hi/lo component, s = sub-chunk of the pair
    xhl = nc.dram_tensor("xhl", [KP, 128, 4, TC], fp8, kind="ExternalInput")
    # (kp, p, s, o)
    nib4 = nc.dram_tensor("nib4", [KP, 128, 2, O], fp8, kind="ExternalInput")
    # (p, kp*2+s, r): cols 0-7 = A_r/scale, 8 = zfrac, 9 = 0
    ae4 = nc.dram_tensor("ae4", [128, KP * 2, 16], fp8, kind="ExternalInput")
    hm = nc.dram_tensor("hm", [16, O], fp16, kind="ExternalInput")
    y = nc.dram_tensor("y", [TC, O], fp16, kind="ExternalOutput")

    with tile.TileContext(nc) as tc:
        with (
            tc.tile_pool(name="nib", bufs=20) as nib_pool,
            tc.tile_pool(name="consts", bufs=1) as const_pool,
            tc.tile_pool(name="xs", bufs=1) as xs_pool,
            tc.tile_pool(name="g", bufs=1) as g_pool,
            tc.tile_pool(name="out", bufs=3) as out_pool,
            tc.tile_pool(name="psum", bufs=8, space="PSUM") as psum_pool,
        ):
            hm_t = const_pool.tile([16, O], fp16, tag="hm")
            nc.sync.dma_start(hm_t[:], hm[:, :])
            ae_t = const_pool.tile([128, KP * 2, 16], fp8, tag="ae")
            nc.sync.dma_start(ae_t[:], ae4[:, :, :])

            # xs tiles: [128, 4, 512] per (token-half, kp); hi = [:,0:2,:],
            # lo = [:,2:4,:]. Resident for the whole kernel.
            xt = [[None] * KP for _ in range(2)]
            gts = [None, None]

            def load_xs(th):
                t0 = th * 512
                for kp in range(KP):
                    x_ = xs_pool.tile([128, 4, 512], fp8, tag=f"x{th}_{kp}",
                                      name=f"x{th}_{kp}")
                    nc.sync.dma_start(x_[:], xhl[kp, :, :, t0:t0 + 512])
                    xt[th][kp] = x_

            def compute_g(th):
                # G for token-half th: psum [10, 512] <- sum over kp pairs of
                # aext^T @ (hi + lo), DoubleRow fp8.
                up = psum_pool.tile([16, 512], fp32, tag="mm", name=f"up{th}")
                for kp in range(KP):
                    ae_s = ae_t[:, kp * 2:(kp + 1) * 2, :]
                    nc.tensor.matmul(up[:], ae_s, xt[th][kp][:, 0:2, :],
                                     start=(kp == 0), stop=False, perf_mode=DR)
                    nc.tensor.matmul(up[:], ae_s, xt[th][kp][:, 2:4, :],
                                     start=False, stop=(kp == KP - 1),
                                     perf_mode=DR)
                gt = g_pool.tile([16, 512], fp16, tag=f"g{th}", name=f"g{th}")
                # row 9 = ALPHA (bias lane; everything in PSUM is ALPHA-scaled)
                nc.vector.memset(gt[:, :], ALPHA)
                nc.vector.tensor_copy(gt[0:9, :], up[0:9, :])
                gts[th] = gt

            def load_nib(h, nib_t):
                o0 = h * OH
                for kp in range(KP):
                    nt = nib_pool.tile([128, 2, OH], fp8, tag="nib",
                                       name=f"nib{h}_{kp}")
                    nc.sync.dma_start(nt[:], nib4[kp, :, :, o0:o0 + OH])
                    nib_t[kp] = nt

            def token_tile(h, tt, nib_t):
                o0 = h * OH
                th, ts = tt // 4, (tt % 4) * 128
                ps = [
                    psum_pool.tile([128, 512], fp32, tag="mm",
                                   name=f"mm{h}_{tt}_{j}")
                    for j in range(4)
                ]
                for kp in range(KP):
                    xh_s = xt[th][kp][:, 0:2, ts:ts + 128]
                    xl_s = xt[th][kp][:, 2:4, ts:ts + 128]
                    for j in range(4):
                        nib_s = nib_t[kp][:, :, j * 512:(j + 1) * 512]
                        nc.tensor.matmul(ps[j][:], xh_s, nib_s,
                                         start=(kp == 0), stop=False,
                                         perf_mode=DR)
                        nc.tensor.matmul(ps[j][:], xl_s, nib_s,
                                         start=False, stop=False,
                                         perf_mode=DR)
                gs = gts[th][:, ts:ts + 128]
                ot = out_pool.tile([128, OH], fp16, tag="out")
                for j in range(4):
                    nc.tensor.matmul(ps[j][:], gs,
                                     hm_t[:, o0 + j * 512:o0 + (j + 1) * 512],
                                     start=False, stop=True)
                    nc.vector.tensor_scalar_mul(
                        ot[:, j * 512:(j + 1) * 512], ps[j][:], 1.0 / ALPHA)
                trow = tt * 128
                nc.scalar.dma_start(y[trow:trow + 128, o0:o0 + OH], ot[:])

            nib_t = [None] * KP
            load_xs(0)
            compute_g(0)
            load_nib(0, nib_t)
            for tt in range(4):
                token_tile(0, tt, nib_t)
            load_xs(1)
            compute_g(1)
            for tt in range(4, 8):
                token_tile(0, tt, nib_t)
            load_nib(1, nib_t)
            for tt in range(8):
                token_tile(1, tt, nib_t)
    nc.compile()
    return nc


def _prep_inputs(x, weight_quant, scale, zero, lora_A, lora_B, bias):
    """Host-side layout prep + sharding. Returns in_maps for 8 cores."""
    import ml_dtypes
    f8 = ml_dtypes.float8_e4m3fn

    scale = np.asarray(scale, np.float32)
    zero = np.asarray(zero, np.float32)

    xs = x.reshape(T, I).astype(np.float32) * (scale[None, :] * ALPHA)
    hi = xs.astype(f8)
    lo = (xs - hi.astype(np.float32)).astype(f8)
    hiT = np.ascontiguousarray(hi.T)   # [I, T]
    loT = np.ascontiguousarray(lo.T)

    zoff = np.rint(zero)
    zfrac = zero - zoff

    wq = weight_quant.astype(np.uint8)            # low byte only is populated
    nib = np.empty((O, I), np.int16)
    nib[:, 0::2] = wq & 15
    nib[:, 1::2] = wq >> 4
    nibz = (nib - zoff.astype(np.int16)[None, :]).astype(f8)   # exact
    # [I, O] -> (kp, s, p, o) -> (kp, p, s, o)
    nib4 = np.ascontiguousarray(
        nibz.T.reshape(KP, 2, 128, O).transpose(0, 2, 1, 3))

    ae = np.zeros((I, 16), np.float32)
    ae[:, 0:8] = (lora_A.astype(np.float32) / scale[None, :]).T
    ae[:, 8] = zfrac
    # [I, 16] -> (kp, s, p, r) -> (p, kp, s, r) -> (p, kp*2+s, r)
    ae4 = np.ascontiguousarray(
        ae.astype(f8).reshape(KP, 2, 128, 16).transpose(2, 0, 1, 3)
    ).reshape(128, KP * 2, 16)

    hmat = np.zeros((16, O), np.float16)
    hmat[0:8, :] = SCALING * lora_B.astype(np.float32).T
    hmat[8, :] = -1.0
    hmat[9, :] = bias
    hmat = np.ascontiguousarray(hmat)

    in_maps = []
    for c in range(NCORES):
        cols = slice(c * TC, (c + 1) * TC)
        # [I, TC] -> (kp, s, p, t) -> (kp, p, hl, s, t) -> (kp, p, hl*2+s, t)
        h4 = hiT[:, cols].reshape(KP, 2, 128, TC).transpose(0, 2, 1, 3)
        l4 = loT[:, cols].reshape(KP, 2, 128, TC).transpose(0, 2, 1, 3)
        xhl = np.ascontiguousarray(
            np.stack([h4, l4], axis=2)).reshape(KP, 128, 4, TC)
        in_maps.append({
            "xhl": xhl,
            "nib4": nib4,
            "ae4": ae4,
            "hm": hmat,
        })
    return in_maps


def run_on_cores(in_maps, trace=False):
    from concourse.bass_utils import run_bass_kernel_spmd

    if "nc" not in _CACHE:
        _CACHE["nc"] = _build_program()
    return run_bass_kernel_spmd(
        _CACHE["nc"], in_maps, list(range(NCORES)), trace=trace
    )


def kernel(x, weight_quant, scale, zero, lora_A, lora_B, bias):
    x = np.asarray(x)
    weight_quant = np.asarray(weight_quant)
    scale = np.asarray(scale, np.float32)
    zero = np.asarray(zero, np.float32)
    lora_A = np.asarray(lora_A, np.float32)
    lora_B = np.asarray(lora_B, np.float32)
    bias = np.asarray(bias, np.float32)

    in_maps = _prep_inputs(x, weight_quant, scale, zero, lora_A, lora_B, bias)
    res = run_on_cores(in_maps).results

    out = np.concatenate([res[c]["y"] for c in range(NCORES)], axis=0)
    return np.ascontiguousarray(out).astype(np.float32).reshape(B, S, O)


# revision 8
# speedup vs baseline: 1.9266x; 1.0374x over previous
"""Quantized (4-bit) LoRA linear for Trainium2, SPMD over 8 NeuronCores.

Math:  y[t,o] = sum_i x[t,i]*W[o,i] + bias[o] + 2.0 * sum_r (x@A^T)[t,r]*B[o,r]
where  W[o,i] = (nib[o,i] - zero[i]) * scale[i],  nib = unpacked 4-bit ints.

Rewrite with xs[t,i] = x[t,i]*scale[i], zoff = round(zero), zfrac = zero-zoff:
  y[t,o] = sum_i xs[t,i]*nib'[o,i]       nib' = nib - zoff in [-15,15], exact fp8
         + sum_k G[t,k]*H[k,o]           K=16 matmul: LoRA + zfrac-corr + bias
  G rows (as (p, s) pairs): (p,0) = u_p = (x@A^T)_p, (0,1) = 1-lane,
  (1,1) = c = sum_i xs*zfrac.  H: (p,0) = 2*B^T rows, (0,1) = bias, (1,1) = -1.

Everything heavy runs in fp8 MatmulPerfMode.DoubleRow (0.5 cycles/row, 2
contraction sub-rows per partition -> 4x fp16 MAC rate). xs is split into
exact fp8 hi+lo components (pre-scaled by ALPHA=256 so both parts avoid
fp8 subnormals; 1/ALPHA is applied at PSUM evacuation). Each 256-channel
pair needs one hi + one lo DoubleRow matmul -> net 2x fewer PE cycles
than an fp16 kernel. G is computed from the hi tiles only (the lo
contribution to G is ~2.5% of terms that are themselves <5% of the
output). The G/H apply is also fp8 DoubleRow: G is evacuated at ALPHA/8
scale into an [8, 2, 512] pair layout (psum rows 0-7 and 32-39 so the
DVE copies start at legal partition bases), H carries the balancing 8x.
Output is written fp16 (upcast on host).

Sharding: 8-way token split (1024 tokens per core), each core computes
the full 4096 outs in two o-half passes; nib' fp8 tiles stream through a
rotating pool, xs stays resident. Startup is DMA-bound, so the first two
token tiles' matmuls (7 of 8 PSUM banks) are emitted interleaved with
the per-pair xs/nib DMAs to keep the PE fed while weights stream in.
"""

import numpy as np

B, S, I, O = 4, 2048, 4096, 4096
T = B * S            # 8192 tokens
NCORES = 8
TC = T // NCORES     # 1024 tokens per core
OH = O // 2          # 2048 outs per o-half pass
KP = I // 256        # 16 contraction pairs (256 channels each)
ALPHA = 256.0        # xs pre-scale so fp8 hi/lo avoid subnormals
GDIV = 8.0           # G evacuated at ALPHA/GDIV; H carries GDIV
SCALING = 2.0        # lora alpha/r

_CACHE = {}


def _build_program():
    import concourse.bacc as bacc
    import concourse.mybir as mybir
    import concourse.tile as tile

    fp16 = mybir.dt.float16
    fp32 = mybir.dt.float32
    fp8 = mybir.dt.float8e4
    DR = mybir.MatmulPerfMode.DoubleRow

    nc = bacc.Bacc("TRN2", target_bir_lowering=False, debug=False)
    # (kp, p, hl*2+s, t): hl = hi/lo component, s = sub-chunk of the pair
    xhl = nc.dram_tensor("xhl", [KP, 128, 4, TC], fp8, kind="ExternalInput")
    # (kp, p, s, o)
    nib4 = nc.dram_tensor("nib4", [KP, 128, 2, O], fp8, kind="ExternalInput")
    # (p, kp*2+s, c): cols 0-7 = A_r/scale, 32 = 0 (1-lane), 33 = zfrac
    ae4 = nc.dram_tensor("ae4", [128, KP * 2, 64], fp8, kind="ExternalInput")
    hm = nc.dram_tensor("hm", [8, 2, O], fp8, kind="ExternalInput")
    y = nc.dram_tensor("y", [TC, O], fp16, kind="ExternalOutput")

    with tile.TileContext(nc) as tc:
        with (
            tc.tile_pool(name="nib", bufs=20) as nib_pool,
            tc.tile_pool(name="consts", bufs=1) as const_pool,
            tc.tile_pool(name="xs", bufs=1) as xs_pool,
            tc.tile_pool(name="g", bufs=1) as g_pool,
            tc.tile_pool(name="out", bufs=4) as out_pool,
            tc.tile_pool(name="psum", bufs=8, space="PSUM") as psum_pool,
        ):
            ae_t = const_pool.tile([128, KP * 2, 64], fp8, tag="ae")
            nc.sync.dma_start(ae_t[:], ae4[:, :, :])
            hm_t = const_pool.tile([8, 2, O], fp8, tag="hm")
            nc.sync.dma_start(hm_t[:], hm[:, :, :])

            # xs tiles: [128, 4, 512] per (token-half, kp); hi = [:,0:2,:],
            # lo = [:,2:4,:]. Resident for the whole kernel.
            xt = [[None] * KP for _ in range(2)]
            gts = [None, None]
            nib_t = [None] * KP

            def dma_xs(th, kp):
                t0 = th * 512
                x_ = xs_pool.tile([128, 4, 512], fp8, tag=f"x{th}_{kp}",
                                  name=f"x{th}_{kp}")
                nc.sync.dma_start(x_[:], xhl[kp, :, :, t0:t0 + 512])
                xt[th][kp] = x_

            def dma_nib(h, kp):
                o0 = h * OH
                nt = nib_pool.tile([128, 2, OH], fp8, tag="nib",
                                   name=f"nib{h}_{kp}")
                nc.sync.dma_start(nt[:], nib4[kp, :, :, o0:o0 + OH])
                nib_t[kp] = nt

            def g_psum(th):
                return psum_pool.tile([64, 512], fp32, tag="mm",
                                      name=f"up{th}")

            def g_mm(up, th, kp):
                nc.tensor.matmul(up[:], ae_t[:, kp * 2:(kp + 1) * 2, :],
                                 xt[th][kp][:, 0:2, :],
                                 start=(kp == 0), stop=(kp == KP - 1),
                                 perf_mode=DR)

            def g_finish(th, up):
                gt = g_pool.tile([8, 2, 512], fp8, tag=f"g{th}",
                                 name=f"g{th}")
                nc.vector.tensor_scalar_mul(gt[:, 0, :], up[0:8, :], 1.0 / GDIV)
                nc.vector.tensor_scalar_mul(gt[:, 1, :], up[32:40, :],
                                            1.0 / GDIV)
                nc.vector.memset(gt[0:1, 1, :], ALPHA / GDIV)  # 1-lane
                gts[th] = gt

            def mm_psum(h, tt, js=range(4)):
                return [
                    psum_pool.tile([128, 512], fp32, tag="mm",
                                   name=f"mm{h}_{tt}_{j}")
                    for j in js
                ]

            def main_mm(h, tt, ps, kp, js):
                th, ts = tt // 4, (tt % 4) * 128
                xh_s = xt[th][kp][:, 0:2, ts:ts + 128]
                xl_s = xt[th][kp][:, 2:4, ts:ts + 128]
                for j in js:
                    nib_s = nib_t[kp][:, :, j * 512:(j + 1) * 512]
                    nc.tensor.matmul(ps[j][:], xh_s, nib_s,
                                     start=(kp == 0), stop=False, perf_mode=DR)
                    nc.tensor.matmul(ps[j][:], xl_s, nib_s,
                                     start=False, stop=False, perf_mode=DR)

            def tail(h, tt, ps):
                # H-apply + evacuate + store, per j so the chain pipelines
                th, ts = tt // 4, (tt % 4) * 128
                o0 = h * OH
                ot = out_pool.tile([128, OH], fp16, tag="out")
                trow = tt * 128
                for j in range(4):
                    nc.tensor.matmul(ps[j][:], gts[th][:, :, ts:ts + 128],
                                     hm_t[:, :, o0 + j * 512:o0 + (j + 1) * 512],
                                     start=False, stop=True, perf_mode=DR)
                    o_s = ot[:, j * 512:(j + 1) * 512]
                    nc.vector.tensor_scalar_mul(o_s, ps[j][:], 1.0 / ALPHA)
                    nc.scalar.dma_start(
                        y[trow:trow + 128,
                          o0 + j * 512:o0 + (j + 1) * 512], o_s)

            # ---- o-half 0, startup: interleave DMAs with tt0 + tt1(j0-2) ----
            ga0 = g_psum(0)
            ps0 = mm_psum(0, 0)
            ps1 = mm_psum(0, 1, range(3))
            for kp in range(KP):
                dma_xs(0, kp)
                dma_nib(0, kp)
                g_mm(ga0, 0, kp)
                main_mm(0, 0, ps0, kp, range(4))
                main_mm(0, 1, ps1, kp, range(3))
            g_finish(0, ga0)
            tail(0, 0, ps0)
            # tt1 j3 catch-up into the bank freed by the G evacuation
            ps1 = ps1 + mm_psum(0, 1, [3])
            for kp in range(KP):
                main_mm(0, 1, ps1, kp, [3])
            tail(0, 1, ps1)
            for tt in (2, 3):
                ps = mm_psum(0, tt)
                for kp in range(KP):
                    main_mm(0, tt, ps, kp, range(4))
                tail(0, tt, ps)
            # ---- token-half 1 xs + its G, then o-half 0 tts 4-7 ----
            for kp in range(KP):
                dma_xs(1, kp)
            ga1 = g_psum(1)
            for kp in range(KP):
                g_mm(ga1, 1, kp)
            g_finish(1, ga1)
            for tt in range(4, 8):
                ps = mm_psum(0, tt)
                for kp in range(KP):
                    main_mm(0, tt, ps, kp, range(4))
                tail(0, tt, ps)
            # ---- o-half 1 ----
            for kp in range(KP):
                dma_nib(1, kp)
            for tt in range(8):
                ps = mm_psum(1, tt)
                for kp in range(KP):
                    main_mm(1, tt, ps, kp, range(4))
                tail(1, tt, ps)
    nc.compile()
    return nc


def _prep_inputs(x, weight_quant, scale, zero, lora_A, lora_B, bias):
    """Host-side layout prep + sharding. Returns in_maps for 8 cores."""
    import ml_dtypes
    f8 = ml_dtypes.float8_e4m3fn

    scale = np.asarray(scale, np.float32)
    zero = np.asarray(zero, np.float32)

    xs = x.reshape(T, I).astype(np.float32) * (scale[None, :] * ALPHA)
    hi = xs.astype(f8)
    lo = (xs - hi.astype(np.float32)).astype(f8)
    hiT = np.ascontiguousarray(hi.T)   # [I, T]
    loT = np.ascontiguousarray(lo.T)

    zoff = np.rint(zero)
    zfrac = zero - zoff

    wq = weight_quant.astype(np.uint8)            # low byte only is populated
    nib = np.empty((O, I), np.int16)
    nib[:, 0::2] = wq & 15
    nib[:, 1::2] = wq >> 4
    nibz = (nib - zoff.astype(np.int16)[None, :]).astype(f8)   # exact
    # [I, O] -> (kp, s, p, o) -> (kp, p, s, o)
    nib4 = np.ascontiguousarray(
        nibz.T.reshape(KP, 2, 128, O).transpose(0, 2, 1, 3))

    ae = np.zeros((I, 64), np.float32)
    ae[:, 0:8] = (lora_A.astype(np.float32) / scale[None, :]).T
    ae[:, 33] = zfrac                  # col 32 stays 0: 1-lane placeholder
    # [I, 64] -> (kp, s, p, c) -> (p, kp, s, c) -> (p, kp*2+s, c)
    ae4 = np.ascontiguousarray(
        ae.astype(f8).reshape(KP, 2, 128, 64).transpose(2, 0, 1, 3)
    ).reshape(128, KP * 2, 64)

    hmat = np.zeros((8, 2, O), np.float32)
    hmat[:, 0, :] = GDIV * SCALING * lora_B.astype(np.float32).T
    hmat[0, 1, :] = GDIV * bias
    hmat[1, 1, :] = -GDIV
    hmat = np.ascontiguousarray(hmat.astype(f8))

    in_maps = []
    for c in range(NCORES):
        cols = slice(c * TC, (c + 1) * TC)
        # [I, TC] -> (kp, s, p, t) -> (kp, p, hl, s, t) -> (kp, p, hl*2+s, t)
        h4 = hiT[:, cols].reshape(KP, 2, 128, TC).transpose(0, 2, 1, 3)
        l4 = loT[:, cols].reshape(KP, 2, 128, TC).transpose(0, 2, 1, 3)
        xhl = np.ascontiguousarray(
            np.stack([h4, l4], axis=2)).reshape(KP, 128, 4, TC)
        in_maps.append({
            "xhl": xhl,
            "nib4": nib4,
            "ae4": ae4,
            "hm": hmat,
        })
    return in_maps


def run_on_cores(in_maps, trace=False):
    from concourse.bass_utils import run_bass_kernel_spmd

    if "nc" not in _CACHE:
        _CACHE["nc"] = _build_program()
    return run_bass_kernel_spmd(
        _CACHE["nc"], in_maps, list(range(NCORES)), trace=trace
    )


def kernel(x, weight_quant, scale, zero, lora_A, lora_B, bias):
    x = np.asarray(x)
    weight_quant = np.asarray(weight_quant)
    scale = np.asarray(scale, np.float32)
    zero = np.asarray(zero, np.float32)
    lora_A = np.asarray(lora_A, np.float32)
    lora_B = np.asarray(lora_B, np.float32)
    bias = np.asarray(bias, np.float32)

    in_maps = _prep_inputs(x, weight_quant, scale, zero, lora_A, lora_B, bias)
    res = run_on_cores(in_maps).results

    out = np.concatenate([res[c]["y"] for c in range(NCORES)], axis=0)
    return np.ascontiguousarray(out).astype(np.float32).reshape(B, S, O)


# revision 10
# speedup vs baseline: 1.9680x; 1.0215x over previous
"""Quantized (4-bit) LoRA linear for Trainium2, SPMD over 8 NeuronCores.

Math:  y[t,o] = sum_i x[t,i]*W[o,i] + bias[o] + 2.0 * sum_r (x@A^T)[t,r]*B[o,r]
where  W[o,i] = (nib[o,i] - zero[i]) * scale[i],  nib = unpacked 4-bit ints.

Rewrite with xs[t,i] = x[t,i]*scale[i], zoff = round(zero), zfrac = zero-zoff:
  y[t,o] = sum_i xs[t,i]*nib'[o,i]       nib' = nib - zoff in [-15,15], exact fp8
         + sum_k G[t,k]*H[k,o]           K=16 matmul: LoRA + zfrac-corr + bias
  G rows (as (p, s) pairs): (p,0) = u_p = (x@A^T)_p, (0,1) = 1-lane,
  (1,1) = c = sum_i xs*zfrac.  H: (p,0) = 2*B^T rows, (0,1) = bias, (1,1) = -1.

Everything heavy runs in fp8 MatmulPerfMode.DoubleRow (0.5 cycles/row, 2
contraction sub-rows per partition -> 4x fp16 MAC rate). xs is split into
exact fp8 hi+lo components (pre-scaled by ALPHA=256 so both parts avoid
fp8 subnormals; 1/ALPHA is applied at PSUM evacuation). Each 256-channel
pair needs one hi + one lo DoubleRow matmul -> net 2x fewer PE cycles
than an fp16 kernel. G is computed from the hi tiles only (the lo
contribution to G is ~2.5% of terms that are themselves <5% of the
output). The G/H apply is also fp8 DoubleRow: G is evacuated at ALPHA/8
scale into an [8, 2, 512] pair layout (psum rows 0-7 and 32-39 so the
DVE copies start at legal partition bases), H carries the balancing 8x.
Output is written fp16 (upcast on host).

Sharding: 8-way token split (1024 tokens per core), each core computes
the full 4096 outs in two o-half passes; nib' fp8 tiles stream through a
rotating pool, xs stays resident.

Scheduling: the kernel start is DMA-bound (xs + nib must land before the
PE can run), so the o-half-0 program is emitted in arrival-rate-matched
waves: phase 1 streams xs + the j0/j1 halves of nib per 256-channel pair
while the PE accumulates G plus seven (token-tile, j) PSUM banks behind
the DMAs; each bank is H-applied/evacuated/stored individually (per-j
early stop) so banks recycle without waiting for full token tiles;
phase 2 streams the j2/j3 nib halves behind the remaining banks. PSUM
evacuations alternate DVE/Act so the final drain chain is short, and
y stores go out per (tile, j) on the Act DMA queue.
"""

import numpy as np

B, S, I, O = 4, 2048, 4096, 4096
T = B * S            # 8192 tokens
NCORES = 8
TC = T // NCORES     # 1024 tokens per core
OH = O // 2          # 2048 outs per o-half pass
KP = I // 256        # 16 contraction pairs (256 channels each)
ALPHA = 256.0        # xs pre-scale so fp8 hi/lo avoid subnormals
GDIV = 8.0           # G evacuated at ALPHA/GDIV; H carries GDIV
SCALING = 2.0        # lora alpha/r

_CACHE = {}


def _build_program():
    import concourse.bacc as bacc
    import concourse.mybir as mybir
    import concourse.tile as tile

    fp16 = mybir.dt.float16
    fp32 = mybir.dt.float32
    fp8 = mybir.dt.float8e4
    DR = mybir.MatmulPerfMode.DoubleRow
    COPY = mybir.ActivationFunctionType.Copy

    nc = bacc.Bacc("TRN2", target_bir_lowering=False, debug=False)
    # (kp, p, hl*2+s, t): hl = hi/lo component, s = sub-chunk of the pair
    xhl = nc.dram_tensor("xhl", [KP, 128, 4, TC], fp8, kind="ExternalInput")
    # (kp, p, s, o)
    nib4 = nc.dram_tensor("nib4", [KP, 128, 2, O], fp8, kind="ExternalInput")
    # (p, kp*2+s, c): cols 0-7 = A_r/scale, 32 = 0 (1-lane), 33 = zfrac
    ae4 = nc.dram_tensor("ae4", [128, KP * 2, 64], fp8, kind="ExternalInput")
    hm = nc.dram_tensor("hm", [8, 2, O], fp8, kind="ExternalInput")
    y = nc.dram_tensor("y", [TC, O], fp16, kind="ExternalOutput")

    with tile.TileContext(nc) as tc:
        with (
            tc.tile_pool(name="nib", bufs=20) as nib_pool,
            tc.tile_pool(name="consts", bufs=1) as const_pool,
            tc.tile_pool(name="xs", bufs=1) as xs_pool,
            tc.tile_pool(name="g", bufs=1) as g_pool,
            tc.tile_pool(name="out", bufs=4) as out_pool,
            tc.tile_pool(name="psum", bufs=8, space="PSUM") as psum_pool,
        ):
            # consts ride the Act DMA queue so the first xs/nib loads on the
            # SP queue aren't serialized behind them
            ae_t = const_pool.tile([128, KP * 2, 64], fp8, tag="ae")
            nc.scalar.dma_start(ae_t[:], ae4[:, :, :])
            hm_t = const_pool.tile([8, 2, O], fp8, tag="hm")
            nc.scalar.dma_start(hm_t[:], hm[:, :, :])

            # xs tiles: [128, 4, 512] per (token-half, kp); hi = [:,0:2,:],
            # lo = [:,2:4,:]. Resident for the whole kernel.
            xt = [[None] * KP for _ in range(2)]
            gts = [None, None]
            nib_t = [None] * KP
            ots = {}

            def dma_xs(th, kp):
                t0 = th * 512
                x_ = xs_pool.tile([128, 4, 512], fp8, tag=f"x{th}_{kp}",
                                  name=f"x{th}_{kp}")
                nc.sync.dma_start(x_[:], xhl[kp, :, :, t0:t0 + 512])
                xt[th][kp] = x_

            def dma_nib(h, kp, c0, c1):
                o0 = h * OH
                if c0 == 0:
                    nib_t[kp] = nib_pool.tile([128, 2, OH], fp8, tag="nib",
                                              name=f"nib{h}_{kp}")
                nc.sync.dma_start(nib_t[kp][:, :, c0:c1],
                                  nib4[kp, :, :, o0 + c0:o0 + c1])

            def g_psum(th):
                return psum_pool.tile([64, 512], fp32, tag="mm",
                                      name=f"up{th}")

            def g_mm(up, th, kp):
                nc.tensor.matmul(up[:], ae_t[:, kp * 2:(kp + 1) * 2, :],
                                 xt[th][kp][:, 0:2, :],
                                 start=(kp == 0), stop=(kp == KP - 1),
                                 perf_mode=DR)

            def g_finish(th, up):
                gt = g_pool.tile([8, 2, 512], fp8, tag=f"g{th}",
                                 name=f"g{th}")
                nc.vector.tensor_scalar_mul(gt[:, 0, :], up[0:8, :], 1.0 / GDIV)
                nc.vector.tensor_scalar_mul(gt[:, 1, :], up[32:40, :],
                                            1.0 / GDIV)
                nc.vector.memset(gt[0:1, 1, :], ALPHA / GDIV)  # 1-lane
                gts[th] = gt

            def mm_bank(h, tt, j):
                return psum_pool.tile([128, 512], fp32, tag="mm",
                                      name=f"mm{h}_{tt}_{j}")

            def main_mm(h, tt, j, ps_j, kp):
                th, ts = tt // 4, (tt % 4) * 128
                nib_s = nib_t[kp][:, :, j * 512:(j + 1) * 512]
                nc.tensor.matmul(ps_j[:], xt[th][kp][:, 0:2, ts:ts + 128],
                                 nib_s, start=(kp == 0), stop=False,
                                 perf_mode=DR)
                nc.tensor.matmul(ps_j[:], xt[th][kp][:, 2:4, ts:ts + 128],
                                 nib_s, start=False, stop=False, perf_mode=DR)

            def tail_j(h, tt, j, ps_j):
                # per-bank H-apply + evacuate + store; banks recycle without
                # waiting for the full token tile
                th, ts = tt // 4, (tt % 4) * 128
                o0 = h * OH
                nc.tensor.matmul(ps_j[:], gts[th][:, :, ts:ts + 128],
                                 hm_t[:, :, o0 + j * 512:o0 + (j + 1) * 512],
                                 start=False, stop=True, perf_mode=DR)
                if (h, tt) not in ots:
                    ots[(h, tt)] = out_pool.tile([128, OH], fp16, tag="out",
                                                 name=f"ot{h}_{tt}")
                o_s = ots[(h, tt)][:, j * 512:(j + 1) * 512]
                if j % 2 == 0:
                    nc.vector.tensor_scalar_mul(o_s, ps_j[:], 1.0 / ALPHA)
                else:
                    nc.scalar.activation(o_s, ps_j[:], COPY, scale=1.0 / ALPHA)
                trow = tt * 128
                nc.scalar.dma_start(
                    y[trow:trow + 128, o0 + j * 512:o0 + (j + 1) * 512], o_s)

            def run_banks(h, pairs, kprange=None, dma=None):
                """Accumulate the given (tt, j) banks, kp-interleaved with
                optional per-kp DMA emission; returns {(tt, j): psum}."""
                ps = {}
                for tt, j in pairs:
                    ps[(tt, j)] = mm_bank(h, tt, j)
                for kp in kprange if kprange is not None else range(KP):
                    if dma is not None:
                        dma(kp)
                    for tt, j in pairs:
                        main_mm(h, tt, j, ps[(tt, j)], kp)
                return ps

            # ---------------- o-half 0 ----------------
            # phase 1: xs-A + nib j0/j1 stream in; PE holds G + 7 banks
            ga0 = g_psum(0)
            P1 = [(0, 0), (0, 1), (1, 0), (1, 1), (2, 0), (2, 1), (3, 0)]
            ps1 = {}
            for tt, j in P1:
                ps1[(tt, j)] = mm_bank(0, tt, j)
            for kp in range(KP):
                dma_xs(0, kp)
                dma_nib(0, kp, 0, 1024)
                g_mm(ga0, 0, kp)
                for tt, j in P1:
                    main_mm(0, tt, j, ps1[(tt, j)], kp)
            g_finish(0, ga0)
            for tt, j in P1:
                tail_j(0, tt, j, ps1[(tt, j)])
            # phase 2: nib j2/j3 halves stream behind the remaining banks;
            # (3,1) is ungated (nib j1 resident) and covers the arrival lag
            for kp in range(KP):
                dma_nib(0, kp, 1024, 2048)
            ps2a = run_banks(0, [(3, 1)])
            P2 = [(0, 2), (0, 3), (1, 2), (1, 3), (2, 2), (2, 3), (3, 2)]
            ps2 = run_banks(0, P2)
            for tt, j in P2:
                tail_j(0, tt, j, ps2[(tt, j)])
            tail_j(0, 3, 1, ps2a[(3, 1)])
            ps2b = run_banks(0, [(3, 3)])
            tail_j(0, 3, 3, ps2b[(3, 3)])
            # token-half B: xs + its G, interleaved with tts 4/5
            ga1 = g_psum(1)
            P3 = [(4, 0), (4, 1), (4, 2), (4, 3), (5, 0), (5, 1), (5, 2)]
            ps3 = {}
            for tt, j in P3:
                ps3[(tt, j)] = mm_bank(0, tt, j)
            for kp in range(KP):
                dma_xs(1, kp)
                g_mm(ga1, 1, kp)
                for tt, j in P3:
                    main_mm(0, tt, j, ps3[(tt, j)], kp)
            g_finish(1, ga1)
            for tt, j in P3:
                tail_j(0, tt, j, ps3[(tt, j)])
            ps3b = run_banks(0, [(5, 3)])
            tail_j(0, 5, 3, ps3b[(5, 3)])
            for tt in (6, 7):
                ps = run_banks(0, [(tt, j) for j in range(4)])
                for j in range(4):
                    tail_j(0, tt, j, ps[(tt, j)])
            # ---------------- o-half 1 ----------------
            for kp in range(KP):
                dma_nib(1, kp, 0, 2048)
            psh = run_banks(1, [(0, j) for j in range(4)] +
                               [(1, j) for j in range(4)])
            for tt in (0, 1):
                for j in range(4):
                    tail_j(1, tt, j, psh[(tt, j)])
            for tt in range(2, 8):
                ps = run_banks(1, [(tt, j) for j in range(4)])
                for j in range(4):
                    tail_j(1, tt, j, ps[(tt, j)])
    nc.compile()
    return nc


def _prep_inputs(x, weight_quant, scale, zero, lora_A, lora_B, bias):
    """Host-side layout prep + sharding. Returns in_maps for 8 cores."""
    import ml_dtypes
    f8 = ml_dtypes.float8_e4m3fn

    scale = np.asarray(scale, np.float32)
    zero = np.asarray(zero, np.float32)

    xs = x.reshape(T, I).astype(np.float32) * (scale[None, :] * ALPHA)
    hi = xs.astype(f8)
    lo = (xs - hi.astype(np.float32)).astype(f8)
    hiT = np.ascontiguousarray(hi.T)   # [I, T]
    loT = np.ascontiguousarray(lo.T)

    zoff = np.rint(zero)
    zfrac = zero - zoff

    wq = weight_quant.astype(np.uint8)            # low byte only is populated
    nib = np.empty((O, I), np.int16)
    nib[:, 0::2] = wq & 15
    nib[:, 1::2] = wq >> 4
    nibz = (nib - zoff.astype(np.int16)[None, :]).astype(f8)   # exact
    # [I, O] -> (kp, s, p, o) -> (kp, p, s, o)
    nib4 = np.ascontiguousarray(
        nibz.T.reshape(KP, 2, 128, O).transpose(0, 2, 1, 3))

    ae = np.zeros((I, 64), np.float32)
    ae[:, 0:8] = (lora_A.astype(np.float32) / scale[None, :]).T
    ae[:, 33] = zfrac                  # col 32 stays 0: 1-lane placeholder
    # [I, 64] -> (kp, s, p, c) -> (p, kp, s, c) -> (p, kp*2+s, c)
    ae4 = np.ascontiguousarray(
        ae.astype(f8).reshape(KP, 2, 128, 64).transpose(2, 0, 1, 3)
    ).reshape(128, KP * 2, 64)

    hmat = np.zeros((8, 2, O), np.float32)
    hmat[:, 0, :] = GDIV * SCALING * lora_B.astype(np.float32).T
    hmat[0, 1, :] = GDIV * bias
    hmat[1, 1, :] = -GDIV
    hmat = np.ascontiguousarray(hmat.astype(f8))

    in_maps = []
    for c in range(NCORES):
        cols = slice(c * TC, (c + 1) * TC)
        # [I, TC] -> (kp, s, p, t) -> (kp, p, hl, s, t) -> (kp, p, hl*2+s, t)
        h4 = hiT[:, cols].reshape(KP, 2, 128, TC).transpose(0, 2, 1, 3)
        l4 = loT[:, cols].reshape(KP, 2, 128, TC).transpose(0, 2, 1, 3)
        xhl = np.ascontiguousarray(
            np.stack([h4, l4], axis=2)).reshape(KP, 128, 4, TC)
        in_maps.append({
            "xhl": xhl,
            "nib4": nib4,
            "ae4": ae4,
            "hm": hmat,
        })
    return in_maps


def run_on_cores(in_maps, trace=False):
    from concourse.bass_utils import run_bass_kernel_spmd

    if "nc" not in _CACHE:
        _CACHE["nc"] = _build_program()
    return run_bass_kernel_spmd(
        _CACHE["nc"], in_maps, list(range(NCORES)), trace=trace
    )


def kernel(x, weight_quant, scale, zero, lora_A, lora_B, bias):
    x = np.asarray(x)
    weight_quant = np.asarray(weight_quant)
    scale = np.asarray(scale, np.float32)
    zero = np.asarray(zero, np.float32)
    lora_A = np.asarray(lora_A, np.float32)
    lora_B = np.asarray(lora_B, np.float32)
    bias = np.asarray(bias, np.float32)

    in_maps = _prep_inputs(x, weight_quant, scale, zero, lora_A, lora_B, bias)
    res = run_on_cores(in_maps).results

    out = np.concatenate([res[c]["y"] for c in range(NCORES)], axis=0)
    return np.ascontiguousarray(out).astype(np.float32).reshape(B, S, O)
